# revision 39
# baseline (speedup 1.0000x reference)
"""GCGRU cell (graph-conv GRU, diffusion-conv gates) on 8 TRN2 NeuronCores.

Math (per batch b, N=1024 nodes, D=2 in-feats, U=64 units, S=2 supports):
  x0   = [H_b | inputs_b]                          (N, 66)  (feature-permuted)
  for gate g in {r, u, c}:
    pre_g = x0g @ Wg_m0 + sum_s A_s @ (x0g @ Wg_{m=s+1}) + bias_g
  (reassociated: (A_s @ x0) @ W == A_s @ (x0 @ W), so the N x N supports
   multiply a tiny (N, 64) matrix instead of the other association order)
  r, u = sigmoid(pre_r), sigmoid(pre_u); c = tanh(pre_c with x0c=[r*H|inputs])
  h = u * H + (1 - u) * c

Implementation notes (v2):
  - Data parallel over batch: 32 batches -> 4 per core, no collectives.
  - supports[b] cast f32->bf16 during the HBM->SBUF DMA (SWDGE) in 2 MB
    chunks (2 per batch) for near-line-rate DMA; chunk DMAs are emitted
    first so the load stream never starves the consumers.
  - A is transposed 128x128 tile-wise on the TensorEngine (transpose-mode
    matmul, bf16 identity moving); four tiles chain into one [128,512]
    bf16 PSUM group.  The PSUM->SBUF drains are round-robined across
    Vector/Scalar/GpSimd so no single engine throttles the PE.
  - Gate pre-activations accumulate in f32 PSUM over 512-column chunks
    with the transposed A_s tiles as the moving operand; sigmoid/tanh run
    on the ScalarEngine straight out of PSUM.
  - h is finalized per 512-column chunk (elementwise + transpose-back +
    store) so the tail after the last A byte is consumed stays short.
"""

import numpy as np

import concourse.bacc as bacc
import concourse.mybir as mybir
import concourse.tile as tile
from concourse.bass_utils import run_bass_kernel_spmd
from concourse.masks import make_identity

B, N, D, U, S = 32, 1024, 2, 64, 2
F = D + U                      # 66
NCORES = 8
BPC = B // NCORES              # 4 batches per core
P = 128                        # partitions
JB = N // P                    # 8 j-blocks per support
F32 = mybir.dt.float32
BF16 = mybir.dt.bfloat16
FP8 = mybir.dt.float8e4
ASCALE = 32.0                  # pre-scale on A so fp8e4 sees ~unit values

_COMPILED = {}


def _build():
    nc = bacc.Bacc("TRN2", target_bir_lowering=False, debug=False)

    t_inputs = nc.dram_tensor("inputs", [BPC, N, D], F32, kind="ExternalInput")
    t_supports = nc.dram_tensor("supports", [BPC, N, N, S], F32, kind="ExternalInput")
    t_hprev = nc.dram_tensor("h_prev", [BPC, N * U], F32, kind="ExternalInput")
    t_wk = {g: nc.dram_tensor(f"{g}_kernel", [F * 3, U], F32, kind="ExternalInput")
            for g in "ruc"}
    t_wb = {g: nc.dram_tensor(f"{g}_bias", [U], F32, kind="ExternalInput")
            for g in "ruc"}
    t_out = nc.dram_tensor("out", [BPC, N * U], F32, kind="ExternalOutput")

    QC = 2                 # i-tiles per load chunk (2 MB f32 per chunk)
    NCH = N // (QC * P)    # 4 chunks per batch
    NQ = N // NCH          # 256-column phase chunks
    JPC = JB // NCH        # j-block pairs per phase chunk

    with tile.TileContext(nc) as tc:
        with (
            tc.tile_pool(name="const", bufs=1) as constp,
            tc.tile_pool(name="wt", bufs=1) as wtp,
            tc.tile_pool(name="abf", bufs=6) as abfp,
            tc.tile_pool(name="at", bufs=2) as atp,
            tc.tile_pool(name="act", bufs=2) as actp,
            tc.tile_pool(name="psA", bufs=2, space="PSUM") as psA,
            tc.tile_pool(name="psB", bufs=2, space="PSUM") as psB,
        ):
            # supports stay f32: a plain DMA runs at ~2x the rate of the
            # casting DMA; the f32->bf16 cast happens in the transpose-PSUM
            # drains instead.
            sup4 = t_supports.ap().rearrange(
                "b (q p) j two -> b p q (j two)", p=P)
            abts = {}

            def load_chunk(b, ch):
                ab = abfp.tile([P, QC * N * S], F32, tag="abf", name="ab",
                               bufs=6)
                nc.gpsimd.dma_start(
                    ab[:], sup4[b, :, ch * QC:(ch + 1) * QC, :])
                abts[(b, ch)] = ab

            # ---- constants ----
            id_bf = constp.tile([P, P], BF16, tag="id_bf")
            make_identity(nc, id_bf[:])
            id_f32 = constp.tile([P, P], F32, tag="id_f32")
            make_identity(nc, id_f32[:])
            # scaled identity: the A transposes are regular matmuls
            # A_tile.T @ (ASCALE * I), folding the fp8 pre-scale in for free
            id32 = constp.tile([P, P], BF16, tag="id32")
            nc.vector.tensor_scalar_mul(id32[:], id_bf[:], ASCALE)

            # ---- gate weights, hop blocks, permuted to [H|inputs], bf16 ----
            # W rows are (f, m) pairs, m fastest: row f*3 + m.  One staging
            # DMA per gate (rows permuted to [H|inputs]), bf16 casts on DVE.
            # weight/bias staging DMAs go on the gpsimd (SWDGE) ring ahead
            # of the supports chunks: FIFO per ring means they complete
            # before the first 2 MB chunk packet, instead of starving in
            # packet round-robin behind it.
            wst = {}
            for g in "ruc":
                st = wtp.tile([F, 3 * U], F32, tag=f"wst_{g}", name=f"wst_{g}")
                src = t_wk[g].ap().rearrange("(f three) u -> f (three u)", three=3)
                nc.gpsimd.dma_start(st[0:U, :], src[D:F, :])
                nc.gpsimd.dma_start(st[U:F, :], src[0:D, :])
                wst[g] = st

            def w_block(g, m):
                return wst[g][:, m * U:(m + 1) * U]

            # m0 weights carry the ASCALE so their psum contribution matches
            # the scaled diffusion terms; the activation divides it back out.
            w0ru = wtp.tile([F, 2 * U], BF16, tag="w0ru")
            nc.vector.tensor_scalar_mul(w0ru[:, 0:U], w_block("r", 0), ASCALE)
            nc.vector.tensor_scalar_mul(w0ru[:, U:2 * U], w_block("u", 0), ASCALE)
            wru_s = []
            for s in range(S):
                w = wtp.tile([F, 2 * U], BF16, tag=f"wru{s}")
                nc.vector.tensor_copy(w[:, 0:U], w_block("r", s + 1))
                nc.vector.tensor_copy(w[:, U:2 * U], w_block("u", s + 1))
                wru_s.append(w)
            wc0 = wtp.tile([F, U], BF16, tag="wc0")
            nc.vector.tensor_scalar_mul(wc0[:], w_block("c", 0), ASCALE)
            wc_s = []
            for s in range(S):
                w = wtp.tile([F, U], BF16, tag=f"wcs{s}")
                nc.vector.tensor_copy(w[:], w_block("c", s + 1))
                wc_s.append(w)

            bias = {}
            for g in "ruc":
                bt = wtp.tile([U, 1], F32, tag=f"bias_{g}")
                nc.gpsimd.dma_start(bt[:], t_wb[g].ap().rearrange("(u one) -> u one", one=1))
                bias[g] = bt

            # ---- x0 loads for every batch, emitted before any supports
            #      chunk.  Layout [p, g, f] with node n = p*8 + g so the
            #      h_prev read is one contiguous 2 KB run per partition
            #      (256 B-descriptor loads starve behind the supports
            #      packets otherwise). ----
            G = N // P             # 8 nodes per partition
            x0ns = []
            for b in range(BPC):
                x0n = actp.tile([P, G * F], F32, tag="x0n", bufs=2,
                                name="x0n")
                nc.sync.dma_start(
                    x0n[:].rearrange("p (g f) -> p g f", f=F)[:, :, 0:U],
                    t_hprev.ap()[b].rearrange("(p g u) -> p g u", p=P, u=U))
                nc.sync.dma_start(
                    x0n[:].rearrange("p (g f) -> p g f", f=F)[:, :, U:F],
                    t_inputs.ap()[b].rearrange("(p g) d -> p g d", p=P))
                x0ns.append(x0n)

            for ch in range(NCH):
                load_chunk(0, ch)

            for b in range(BPC):
                # prefetch next batch's chunks (abf bufs recycle as batch
                # b's chunks are consumed)
                if b + 1 < BPC:
                    for ch in range(NCH):
                        load_chunk(b + 1, ch)

                at = [atp.tile([P, JB * N], FP8, tag=f"at{s}", name=f"at{s}")
                      for s in range(S)]
                at3 = [a[:].rearrange("p (jb i) -> p jb i", i=N) for a in at]

                x0n = x0ns[b]
                x0T = actp.tile([F, N], F32, tag="x0T")
                x0Tb = actp.tile([F, N], BF16, tag="x0Tb")
                # transpose block g holds nodes {p*8 + g}; the drain scatters
                # them to natural column order with a stride-8 free AP
                x0T_g = x0T[:].rearrange("f (p g) -> f g p", g=G)
                x0Tb_g = x0Tb[:].rearrange("f (p g) -> f g p", g=G)
                for g2 in range(G // 2):
                    px = psA.tile([F, 2 * P], F32, tag="psAx", bufs=2)
                    for q in range(2):
                        nc.tensor.matmul(
                            px[:, q * P:(q + 1) * P],
                            x0n[:, (2 * g2 + q) * F:(2 * g2 + q + 1) * F],
                            id_f32[:], start=(q == 0), stop=(q == 1),
                            is_transpose=True)
                    px3 = px[:].rearrange("f (g p) -> f g p", g=2)
                    nc.scalar.copy(x0T_g[:, 2 * g2:2 * g2 + 2, :], px3)
                    nc.scalar.copy(x0Tb_g[:, 2 * g2:2 * g2 + 2, :], px3)

                # ---- Z_ru_s = x0 @ [Wr_{s+1} | Wu_{s+1}]  (N, 128) fp8 ----
                zru = []
                for s in range(S):
                    z = actp.tile([P, JB * 2 * U], FP8, tag=f"zru{s}")
                    for jb2 in range(JB // 2):
                        pz = psA.tile([P, 2 * 2 * U], F32, tag="psAx", bufs=2)
                        for q in range(2):
                            nc.tensor.matmul(
                                pz[:, q * 2 * U:(q + 1) * 2 * U],
                                x0Tb[:, (2 * jb2 + q) * P:(2 * jb2 + q + 1) * P],
                                wru_s[s][:], start=(q == 0), stop=(q == 1))
                        nc.vector.tensor_copy(
                            z[:, jb2 * 4 * U:(jb2 + 1) * 4 * U], pz[:])
                    zru.append(z)
                zru3 = [z[:].rearrange("p (jb c) -> p jb c", c=2 * U)
                        for z in zru]

                # ---- supports chunk transposes: four 128x128 tiles chain in
                #      one [128, 512] bf16 PSUM group; drains round-robin over
                #      Vector/Scalar/GpSimd ----
                def load_and_transpose(ch):
                    ab = abts.pop((b, ch))
                    # bf16 view of the f32 chunk: the hi halves (h=1) ARE the
                    # bf16 truncation of A, so the PE streams at bf16 rate
                    # straight from the f32 load.  Regular matmul against the
                    # scaled identity gives ASCALE*A^T in f32 psum, which the
                    # drain rounds to fp8.
                    ab5 = ab[:].bitcast(BF16).rearrange(
                        "p (q j s h) -> p q j s h", q=QC, s=S, h=2)
                    k = 0
                    for s in range(S):
                        for jb in range(JB):
                            pt4 = psA.tile([P, QC * P], F32, tag="psA",
                                           bufs=2, name="pt4")
                            for q in range(QC):
                                nc.tensor.matmul(
                                    pt4[:, q * P:(q + 1) * P],
                                    ab5[:, q, jb * P:(jb + 1) * P, s, 1],
                                    id32[:],
                                    start=(q == 0), stop=(q == QC - 1))
                            dst = at[s][:, jb * N + ch * QC * P:
                                        jb * N + (ch + 1) * QC * P]
                            if k % 4 == 3:
                                nc.scalar.copy(dst, pt4[:])
                            else:
                                nc.vector.tensor_copy(dst, pt4[:])
                            k += 1

                rT = actp.tile([U, N], BF16, tag="rT")
                uT = actp.tile([U, N], F32, tag="uT")

                def phase1(ic):
                    # column chunk: only needs chunk ic's transposes.  fp8
                    # DoubleRow contracts two j-blocks per matmul.
                    p1 = psB.tile([P, NQ], F32, tag="psB", name="p1")
                    k = 0
                    for s in range(S):
                        for t in range(JB // 2):
                            nc.tensor.matmul(
                                p1[:],
                                zru3[s][:, 2 * t:2 * t + 2, :],
                                at3[s][:, 2 * t:2 * t + 2,
                                       ic * NQ:(ic + 1) * NQ],
                                start=(k == 0), stop=False,
                                perf_mode=mybir.MatmulPerfMode.DoubleRow)
                            k += 1
                    nc.tensor.matmul(
                        p1[:], w0ru[:], x0Tb[:, ic * NQ:(ic + 1) * NQ],
                        start=False, stop=True)
                    nc.scalar.activation(
                        rT[:, ic * NQ:(ic + 1) * NQ], p1[0:U, :],
                        mybir.ActivationFunctionType.Sigmoid,
                        bias=bias["r"][:], scale=1.0 / ASCALE)
                    nc.scalar.activation(
                        uT[:, ic * NQ:(ic + 1) * NQ], p1[U:2 * U, :],
                        mybir.ActivationFunctionType.Sigmoid,
                        bias=bias["u"][:], scale=1.0 / ASCALE)

                for ch in range(NCH):
                    load_and_transpose(ch)
                    phase1(ch)

                # ---- x0c^T = [(r * H)^T | inputs^T] (bf16) ----
                x0cT = actp.tile([F, N], BF16, tag="x0cT")
                nc.vector.tensor_copy(x0cT[U:F, :], x0Tb[U:F, :])
                for jb in range(JB):
                    nc.vector.tensor_mul(
                        x0cT[0:U, jb * P:(jb + 1) * P],
                        rT[:, jb * P:(jb + 1) * P],
                        x0T[0:U, jb * P:(jb + 1) * P])

                # ---- Z_c_s = x0c @ Wc_{s+1}  (N, 64) fp8 ----
                zc = []
                for s in range(S):
                    z = actp.tile([P, JB * U], FP8, tag=f"zc{s}")
                    for jb2 in range(JB // 2):
                        pz = psA.tile([P, 2 * U], F32, tag="psAx", bufs=2)
                        for q in range(2):
                            nc.tensor.matmul(
                                pz[:, q * U:(q + 1) * U],
                                x0cT[:, (2 * jb2 + q) * P:(2 * jb2 + q + 1) * P],
                                wc_s[s][:], start=(q == 0), stop=(q == 1))
                        nc.vector.tensor_copy(
                            z[:, jb2 * 2 * U:(jb2 + 1) * 2 * U], pz[:])
                    zc.append(z)
                zc3 = [z[:].rearrange("p (jb c) -> p jb c", c=U) for z in zc]

                # ---- phase 2 + h finalization, per 512-column chunk ----
                cT = actp.tile([U, N], F32, tag="cT")
                hT = actp.tile([U, N], F32, tag="hT")
                hnat = actp.tile([P, JB * U], F32, tag="hnat")
                for ic in range(NCH):
                    p2 = psB.tile([U, NQ], F32, tag="psB2", name="p2")
                    k = 0
                    for s in range(S):
                        for t in range(JB // 2):
                            nc.tensor.matmul(
                                p2[:],
                                zc3[s][:, 2 * t:2 * t + 2, :],
                                at3[s][:, 2 * t:2 * t + 2,
                                       ic * NQ:(ic + 1) * NQ],
                                start=(k == 0), stop=False,
                                perf_mode=mybir.MatmulPerfMode.DoubleRow)
                            k += 1
                    nc.tensor.matmul(
                        p2[:], wc0[:], x0cT[:, ic * NQ:(ic + 1) * NQ],
                        start=False, stop=True)
                    cs = slice(ic * NQ, (ic + 1) * NQ)
                    nc.scalar.activation(
                        cT[:, cs], p2[:],
                        mybir.ActivationFunctionType.Tanh,
                        bias=bias["c"][:], scale=1.0 / ASCALE)
                    # h^T = c^T + u^T * (H^T - c^T)
                    nc.vector.tensor_sub(hT[:, cs], x0T[0:U, cs], cT[:, cs])
                    nc.vector.tensor_mul(hT[:, cs], hT[:, cs], uT[:, cs])
                    nc.vector.tensor_add(hT[:, cs], hT[:, cs], cT[:, cs])
                    for jb2 in range(ic * JPC // 2, (ic + 1) * JPC // 2):
                        ph = psA.tile([P, 2 * U], F32, tag="psAx", bufs=2)
                        for q in range(2):
                            nc.tensor.matmul(
                                ph[:, q * U:(q + 1) * U],
                                hT[:, (2 * jb2 + q) * P:(2 * jb2 + q + 1) * P],
                                id_f32[0:U, 0:U], start=(q == 0), stop=(q == 1),
                                is_transpose=True)
                        nc.vector.tensor_copy(
                            hnat[:, jb2 * 2 * U:(jb2 + 1) * 2 * U], ph[:])
                    nc.sync.dma_start(
                        t_out.ap()[b].rearrange(
                            "(jb p u) -> p jb u", p=P, u=U)[:, ic * JPC:(ic + 1) * JPC, :],
                        hnat[:].rearrange(
                            "p (jb u) -> p jb u", u=U)[:, ic * JPC:(ic + 1) * JPC, :])

    nc.finalize()
    return nc


def _make_in_maps(inputs):
    in_maps = []
    for c in range(NCORES):
        lo, hi = c * BPC, (c + 1) * BPC
        in_maps.append({
            "inputs": np.ascontiguousarray(inputs["inputs"][lo:hi], np.float32),
            "supports": np.ascontiguousarray(inputs["supports"][lo:hi], np.float32),
            "h_prev": np.ascontiguousarray(inputs["h_prev"][lo:hi], np.float32),
            "r_kernel": np.ascontiguousarray(inputs["r_kernel"], np.float32),
            "u_kernel": np.ascontiguousarray(inputs["u_kernel"], np.float32),
            "c_kernel": np.ascontiguousarray(inputs["c_kernel"], np.float32),
            "r_bias": np.ascontiguousarray(inputs["r_bias"], np.float32),
            "u_bias": np.ascontiguousarray(inputs["u_bias"], np.float32),
            "c_bias": np.ascontiguousarray(inputs["c_bias"], np.float32),
        })
    return in_maps


def kernel(**inputs):
    nc = _COMPILED.get("nc")
    if nc is None:
        nc = _COMPILED["nc"] = _build()

    in_maps = _make_in_maps(inputs)
    last_err = None
    for _ in range(3):
        try:
            res = run_bass_kernel_spmd(nc, in_maps, core_ids=list(range(NCORES)))
            out = np.concatenate(
                [np.asarray(res.results[c]["out"]) for c in range(NCORES)], axis=0)
            return out.astype(np.float32)
        except Exception as e:  # sporadic NRT_EXEC_UNIT_UNRECOVERABLE flakes
            last_err = e
    raise last_err


# revision 40
# speedup vs baseline: 1.2870x; 1.2870x over previous
"""GCGRU cell (graph-conv GRU, diffusion-conv gates) on 8 TRN2 NeuronCores.

Math (per batch b, N=1024 nodes, D=2 in-feats, U=64 units, S=2 supports):
  x0   = [H_b | inputs_b]                          (N, 66)  (feature-permuted)
  for gate g in {r, u, c}:
    pre_g = x0g @ Wg_m0 + sum_s A_s @ (x0g @ Wg_{m=s+1}) + bias_g
  (reassociated: (A_s @ x0) @ W == A_s @ (x0 @ W), so the N x N supports
   multiply a tiny (N, 64) matrix instead of the other association order)
  r, u = sigmoid(pre_r), sigmoid(pre_u); c = tanh(pre_c with x0c=[r*H|inputs])
  h = u * H + (1 - u) * c

Implementation notes:
  - Data parallel over batch: 32 batches -> 4 per core, no collectives.
  - supports are loaded as RAW f32 (a plain DMA runs ~2x the rate of the
    casting DMA).  The PE transposes the HI bf16 halves of the f32 words
    (bitcast view, stride-4), which IS the bf16 truncation of A, so the
    transposes stream at bf16 rate straight out of the f32 chunks.
  - Weight/bias staging DMAs ride the gpsimd SWDGE ring ahead of the
    supports chunks (FIFO per ring => no packet starvation); h_prev is
    loaded contiguously (node = p*8+g layout) and the x0 transposes
    scatter to natural order with a stride-8 drain AP.
  - x0 transposes and z_ru for ALL batches run up front, inside the
    initial chunk-DMA wait window, so batch boundaries stay gap-free.
  - Gate pre-activations accumulate in f32 PSUM over 256-column chunks
    with the transposed A_s tiles as the moving operand; sigmoid/tanh on
    the ScalarEngine straight out of PSUM; transpose drains split
    DVE/Scalar 3:1.
  - h is finalized per 256-column chunk, with the transpose-back matmuls
    emitted one phase-2 chunk behind so the PE never waits on the DVE
    elementwise chain.
"""

import numpy as np

import concourse.bacc as bacc
import concourse.mybir as mybir
import concourse.tile as tile
from concourse.bass_utils import run_bass_kernel_spmd
from concourse.masks import make_identity

B, N, D, U, S = 32, 1024, 2, 64, 2
F = D + U                      # 66
NCORES = 8
BPC = B // NCORES              # 4 batches per core
P = 128                        # partitions
JB = N // P                    # 8 j-blocks per support
G = N // P                     # 8 nodes per partition (x0 layout)
F32 = mybir.dt.float32
BF16 = mybir.dt.bfloat16

_COMPILED = {}


def _build():
    nc = bacc.Bacc("TRN2", target_bir_lowering=False, debug=False)

    t_inputs = nc.dram_tensor("inputs", [BPC, N, D], F32, kind="ExternalInput")
    t_supports = nc.dram_tensor("supports", [BPC, N, N, S], F32, kind="ExternalInput")
    t_hprev = nc.dram_tensor("h_prev", [BPC, N * U], F32, kind="ExternalInput")
    t_wk = {g: nc.dram_tensor(f"{g}_kernel", [F * 3, U], F32, kind="ExternalInput")
            for g in "ruc"}
    t_wb = {g: nc.dram_tensor(f"{g}_bias", [U], F32, kind="ExternalInput")
            for g in "ruc"}
    t_out = nc.dram_tensor("out", [BPC, N * U], F32, kind="ExternalOutput")

    QC = 2                 # i-tiles per load chunk (2 MB f32 per chunk)
    NCH = N // (QC * P)    # 4 chunks per batch
    NQ = N // NCH          # 256-column phase chunks
    JPC = JB // NCH        # j-blocks per phase chunk

    with tile.TileContext(nc) as tc:
        with (
            tc.tile_pool(name="const", bufs=1) as constp,
            tc.tile_pool(name="wt", bufs=1) as wtp,
            tc.tile_pool(name="abf", bufs=3) as abfp,
            tc.tile_pool(name="at", bufs=2) as atp,
            tc.tile_pool(name="act", bufs=2) as actp,
            tc.tile_pool(name="psA", bufs=2, space="PSUM") as psA,
            tc.tile_pool(name="psB", bufs=2, space="PSUM") as psB,
        ):
            sup4 = t_supports.ap().rearrange(
                "b (q p) j two -> b p q (j two)", p=P)
            abts = {}

            def load_chunk(b, ch):
                ab = abfp.tile([P, QC * N * S], F32, tag="abf", name="ab",
                               bufs=3)
                nc.gpsimd.dma_start(
                    ab[:], sup4[b, :, ch * QC:(ch + 1) * QC, :])
                abts[(b, ch)] = ab

            # ---- constants ----
            id_bf = constp.tile([P, P], BF16, tag="id_bf")
            make_identity(nc, id_bf[:])
            id_f32 = constp.tile([P, P], F32, tag="id_f32")
            make_identity(nc, id_f32[:])

            # ---- gate weights staged via the gpsimd ring (ahead of the
            #      supports chunks in FIFO order) ----
            wst = {}
            for g in "ruc":
                st = wtp.tile([F, 3 * U], F32, tag=f"wst_{g}", name=f"wst_{g}")
                src = t_wk[g].ap().rearrange("(f three) u -> f (three u)", three=3)
                nc.gpsimd.dma_start(st[0:U, :], src[D:F, :])
                nc.gpsimd.dma_start(st[U:F, :], src[0:D, :])
                wst[g] = st

            def w_block(g, m):
                return wst[g][:, m * U:(m + 1) * U]

            w0ru = wtp.tile([F, 2 * U], BF16, tag="w0ru")
            nc.vector.tensor_copy(w0ru[:, 0:U], w_block("r", 0))
            nc.vector.tensor_copy(w0ru[:, U:2 * U], w_block("u", 0))
            wru_s = []
            for s in range(S):
                w = wtp.tile([F, 2 * U], BF16, tag=f"wru{s}")
                nc.vector.tensor_copy(w[:, 0:U], w_block("r", s + 1))
                nc.vector.tensor_copy(w[:, U:2 * U], w_block("u", s + 1))
                wru_s.append(w)
            wc0 = wtp.tile([F, U], BF16, tag="wc0")
            nc.vector.tensor_copy(wc0[:], w_block("c", 0))
            wc_s = []
            for s in range(S):
                w = wtp.tile([F, U], BF16, tag=f"wcs{s}")
                nc.vector.tensor_copy(w[:], w_block("c", s + 1))
                wc_s.append(w)

            bias = {}
            for g in "ruc":
                bt = wtp.tile([U, 1], F32, tag=f"bias_{g}")
                nc.gpsimd.dma_start(bt[:], t_wb[g].ap().rearrange("(u one) -> u one", one=1))
                bias[g] = bt

            # ---- x0 loads for every batch (contiguous h_prev reads) ----
            x0ns = []
            for b in range(BPC):
                x0n = actp.tile([P, G * F], F32, tag="x0n", bufs=BPC,
                                name="x0n")
                nc.sync.dma_start(
                    x0n[:].rearrange("p (g f) -> p g f", f=F)[:, :, 0:U],
                    t_hprev.ap()[b].rearrange("(p g u) -> p g u", p=P, u=U))
                nc.sync.dma_start(
                    x0n[:].rearrange("p (g f) -> p g f", f=F)[:, :, U:F],
                    t_inputs.ap()[b].rearrange("(p g) d -> p g d", p=P))
                x0ns.append(x0n)

            for ch in range(NCH):
                load_chunk(0, ch)

            # ---- x0 transposes + z_ru for ALL batches, up front: this PE
            #      work fills the initial chunk-DMA wait window and keeps
            #      batch boundaries gap-free later ----
            x0Tbs, zrus = [], []
            for b in range(BPC):
                x0Tb = actp.tile([F, N], BF16, tag="x0Tb", bufs=BPC,
                                 name="x0Tb")
                x0Tb_g = x0Tb[:].rearrange("f (p g) -> f g p", g=G)
                for g2 in range(G // 2):
                    px = psA.tile([F, 2 * P], F32, tag="psAx", bufs=2)
                    for q in range(2):
                        nc.tensor.matmul(
                            px[:, q * P:(q + 1) * P],
                            x0ns[b][:, (2 * g2 + q) * F:(2 * g2 + q + 1) * F],
                            id_f32[:], start=(q == 0), stop=(q == 1),
                            is_transpose=True)
                    px3 = px[:].rearrange("f (g p) -> f g p", g=2)
                    nc.scalar.copy(x0Tb_g[:, 2 * g2:2 * g2 + 2, :], px3)
                x0Tbs.append(x0Tb)

                zru = []
                for s in range(S):
                    z = actp.tile([P, JB * 2 * U], BF16, tag=f"zru{s}",
                                  bufs=BPC, name="zru")
                    for jb2 in range(JB // 2):
                        pz = psA.tile([P, 2 * 2 * U], F32, tag="psAx", bufs=2)
                        for q in range(2):
                            nc.tensor.matmul(
                                pz[:, q * 2 * U:(q + 1) * 2 * U],
                                x0Tb[:, (2 * jb2 + q) * P:(2 * jb2 + q + 1) * P],
                                wru_s[s][:], start=(q == 0), stop=(q == 1))
                        nc.scalar.copy(
                            z[:, jb2 * 4 * U:(jb2 + 1) * 4 * U], pz[:])
                    zru.append(z)
                zrus.append(zru)

            for b in range(BPC):
                if b + 1 < BPC:
                    for ch in range(NCH):
                        load_chunk(b + 1, ch)

                at = [atp.tile([P, JB * N], BF16, tag=f"at{s}", name=f"at{s}")
                      for s in range(S)]
                x0Tb = x0Tbs[b]
                zru = zrus[b]

                def load_and_transpose(ch):
                    ab = abts.pop((b, ch))
                    # bf16 view of the f32 chunk: the hi halves (h=1) ARE the
                    # bf16 truncation of A, so the PE streams at bf16 rate
                    # straight from the f32 load.
                    ab5 = ab[:].bitcast(BF16).rearrange(
                        "p (q j s h) -> p q j s h", q=QC, s=S, h=2)
                    k = 0
                    for s in range(S):
                        for jb in range(JB):
                            pt4 = psA.tile([P, QC * P], BF16, tag="psA",
                                           bufs=2, name="pt4")
                            for q in range(QC):
                                nc.tensor.matmul(
                                    pt4[:, q * P:(q + 1) * P],
                                    ab5[:, q, jb * P:(jb + 1) * P, s, 1],
                                    id_bf[:], is_transpose=True,
                                    start=(q == 0), stop=(q == QC - 1))
                            dst = at[s][:, jb * N + ch * QC * P:
                                        jb * N + (ch + 1) * QC * P]
                            if k % 4 == 3:
                                nc.scalar.copy(dst, pt4[:])
                            else:
                                nc.vector.tensor_copy(dst, pt4[:])
                            k += 1

                rT = actp.tile([U, N], BF16, tag="rT")
                uT = actp.tile([U, N], F32, tag="uT")

                def phase1(ic):
                    # column chunk: only needs chunk ic's transposes
                    p1 = psB.tile([P, NQ], F32, tag="psB", name="p1")
                    k = 0
                    for s in range(S):
                        for jb in range(JB):
                            nc.tensor.matmul(
                                p1[:],
                                zru[s][:, jb * 2 * U:(jb + 1) * 2 * U],
                                at[s][:, jb * N + ic * NQ: jb * N + (ic + 1) * NQ],
                                start=(k == 0), stop=False)
                            k += 1
                    nc.tensor.matmul(
                        p1[:], w0ru[:], x0Tb[:, ic * NQ:(ic + 1) * NQ],
                        start=False, stop=True)
                    nc.scalar.activation(
                        rT[:, ic * NQ:(ic + 1) * NQ], p1[0:U, :],
                        mybir.ActivationFunctionType.Sigmoid, bias=bias["r"][:])
                    nc.scalar.activation(
                        uT[:, ic * NQ:(ic + 1) * NQ], p1[U:2 * U, :],
                        mybir.ActivationFunctionType.Sigmoid, bias=bias["u"][:])

                for ch in range(NCH):
                    load_and_transpose(ch)
                    phase1(ch)

                # ---- x0c^T = [(r * H)^T | inputs^T] (bf16) ----
                x0cT = actp.tile([F, N], BF16, tag="x0cT")
                nc.vector.tensor_copy(x0cT[U:F, :], x0Tb[U:F, :])
                for jb in range(JB):
                    nc.vector.tensor_mul(
                        x0cT[0:U, jb * P:(jb + 1) * P],
                        rT[:, jb * P:(jb + 1) * P],
                        x0Tb[0:U, jb * P:(jb + 1) * P])

                # ---- Z_c_s = x0c @ Wc_{s+1}  (N, 64) bf16 ----
                zc = []
                for s in range(S):
                    z = actp.tile([P, JB * U], BF16, tag=f"zc{s}")
                    for jb2 in range(JB // 2):
                        pz = psA.tile([P, 2 * U], F32, tag="psAx", bufs=2)
                        for q in range(2):
                            nc.tensor.matmul(
                                pz[:, q * U:(q + 1) * U],
                                x0cT[:, (2 * jb2 + q) * P:(2 * jb2 + q + 1) * P],
                                wc_s[s][:], start=(q == 0), stop=(q == 1))
                        nc.vector.tensor_copy(
                            z[:, jb2 * 2 * U:(jb2 + 1) * 2 * U], pz[:])
                    zc.append(z)

                # ---- phase 2 + h finalization; the transpose-back matmuls
                #      trail one chunk behind so the PE never waits on the
                #      DVE h-chain ----
                cT = actp.tile([U, N], F32, tag="cT")
                hT = actp.tile([U, N], F32, tag="hT")
                hnat = actp.tile([P, JB * U], F32, tag="hnat")

                def hfin(ic):
                    # h^T = c^T + u^T * (H^T - c^T);  back to natural + store
                    cs = slice(ic * NQ, (ic + 1) * NQ)
                    nc.vector.tensor_sub(hT[:, cs], x0Tb[0:U, cs], cT[:, cs])
                    nc.vector.tensor_mul(hT[:, cs], hT[:, cs], uT[:, cs])
                    nc.vector.tensor_add(hT[:, cs], hT[:, cs], cT[:, cs])

                def hback(ic):
                    for jb2 in range(ic * JPC // 2, (ic + 1) * JPC // 2):
                        ph = psA.tile([P, 2 * U], F32, tag="psAx", bufs=2)
                        for q in range(2):
                            nc.tensor.matmul(
                                ph[:, q * U:(q + 1) * U],
                                hT[:, (2 * jb2 + q) * P:(2 * jb2 + q + 1) * P],
                                id_f32[0:U, 0:U], start=(q == 0), stop=(q == 1),
                                is_transpose=True)
                        nc.vector.tensor_copy(
                            hnat[:, jb2 * 2 * U:(jb2 + 1) * 2 * U], ph[:])
                    nc.sync.dma_start(
                        t_out.ap()[b].rearrange(
                            "(jb p u) -> p jb u", p=P, u=U)[:, ic * JPC:(ic + 1) * JPC, :],
                        hnat[:].rearrange(
                            "p (jb u) -> p jb u", u=U)[:, ic * JPC:(ic + 1) * JPC, :])

                def phase2(ic):
                    p2 = psB.tile([U, NQ], F32, tag="psB2", name="p2")
                    k = 0
                    for s in range(S):
                        for jb in range(JB):
                            nc.tensor.matmul(
                                p2[:],
                                zc[s][:, jb * U:(jb + 1) * U],
                                at[s][:, jb * N + ic * NQ: jb * N + (ic + 1) * NQ],
                                start=(k == 0), stop=False)
                            k += 1
                    nc.tensor.matmul(
                        p2[:], wc0[:], x0cT[:, ic * NQ:(ic + 1) * NQ],
                        start=False, stop=True)
                    nc.scalar.activation(
                        cT[:, ic * NQ:(ic + 1) * NQ], p2[:],
                        mybir.ActivationFunctionType.Tanh, bias=bias["c"][:])
                    hfin(ic)

                for ic in range(NCH):
                    phase2(ic)
                    if ic > 0:
                        hback(ic - 1)
                hback(NCH - 1)

    nc.finalize()
    return nc


def _make_in_maps(inputs):
    in_maps = []
    for c in range(NCORES):
        lo, hi = c * BPC, (c + 1) * BPC
        in_maps.append({
            "inputs": np.ascontiguousarray(inputs["inputs"][lo:hi], np.float32),
            "supports": np.ascontiguousarray(inputs["supports"][lo:hi], np.float32),
            "h_prev": np.ascontiguousarray(inputs["h_prev"][lo:hi], np.float32),
            "r_kernel": np.ascontiguousarray(inputs["r_kernel"], np.float32),
            "u_kernel": np.ascontiguousarray(inputs["u_kernel"], np.float32),
            "c_kernel": np.ascontiguousarray(inputs["c_kernel"], np.float32),
            "r_bias": np.ascontiguousarray(inputs["r_bias"], np.float32),
            "u_bias": np.ascontiguousarray(inputs["u_bias"], np.float32),
            "c_bias": np.ascontiguousarray(inputs["c_bias"], np.float32),
        })
    return in_maps


def kernel(**inputs):
    nc = _COMPILED.get("nc")
    if nc is None:
        nc = _COMPILED["nc"] = _build()

    in_maps = _make_in_maps(inputs)
    last_err = None
    for _ in range(3):
        try:
            res = run_bass_kernel_spmd(nc, in_maps, core_ids=list(range(NCORES)))
            out = np.concatenate(
                [np.asarray(res.results[c]["out"]) for c in range(NCORES)], axis=0)
            return out.astype(np.float32)
        except Exception as e:  # sporadic NRT_EXEC_UNIT_UNRECOVERABLE flakes
            last_err = e
    raise last_err


# revision 43
# speedup vs baseline: 1.4478x; 1.1249x over previous
"""GCGRU cell (graph-conv GRU, diffusion-conv gates) on 8 TRN2 NeuronCores.

Math (per batch b, N=1024 nodes, D=2 in-feats, U=64 units, S=2 supports):
  x0   = [H_b | inputs_b]                          (N, 66)  (feature-permuted)
  for gate g in {r, u, c}:
    pre_g = x0g @ Wg_m0 + sum_s A_s @ (x0g @ Wg_{m=s+1}) + bias_g
  (reassociated: (A_s @ x0) @ W == A_s @ (x0 @ W), so the N x N supports
   multiply a tiny (N, 64) matrix instead of the other association order)
  r, u = sigmoid(pre_r), sigmoid(pre_u); c = tanh(pre_c with x0c=[r*H|inputs])
  h = u * H + (1 - u) * c

Implementation notes:
  - Data parallel over batch: 32 batches -> 4 per core, no collectives.
  - supports are loaded as RAW f32 (a plain DMA runs ~2x the rate of the
    casting DMA).  The PE transposes the HI bf16 halves of the f32 words
    (bitcast view, stride-4), which IS the bf16 truncation of A, so the
    transposes stream at bf16 rate straight out of the f32 chunks.
  - Weight/bias staging DMAs ride the gpsimd SWDGE ring ahead of the
    supports chunks (FIFO per ring => no packet starvation); h_prev is
    loaded contiguously (node = p*8+g layout) and the x0 transposes
    scatter to natural order with a stride-8 drain AP.
  - x0 transposes and z_ru for ALL batches run up front, inside the
    initial chunk-DMA wait window, so batch boundaries stay gap-free.
  - Gate pre-activations accumulate in f32 PSUM over 256-column chunks
    with the transposed A_s tiles as the moving operand; sigmoid/tanh on
    the ScalarEngine straight out of PSUM; transpose drains split
    DVE/Scalar 3:1.
  - h is finalized per 256-column chunk, with the transpose-back matmuls
    emitted one phase-2 chunk behind so the PE never waits on the DVE
    elementwise chain.
"""

import numpy as np

import concourse.bacc as bacc
import concourse.mybir as mybir
import concourse.tile as tile
from concourse.bass_utils import run_bass_kernel_spmd
from concourse.masks import make_identity

B, N, D, U, S = 32, 1024, 2, 64, 2
F = D + U                      # 66
NCORES = 8
BPC = B // NCORES              # 4 batches per core
P = 128                        # partitions
JB = N // P                    # 8 j-blocks per support
G = N // P                     # 8 nodes per partition (x0 layout)
F32 = mybir.dt.float32
BF16 = mybir.dt.bfloat16

_COMPILED = {}


def _build():
    nc = bacc.Bacc("TRN2", target_bir_lowering=False, debug=False)

    t_inputs = nc.dram_tensor("inputs", [BPC, N, D], F32, kind="ExternalInput")
    t_supports = nc.dram_tensor("supports", [BPC, N, N, S], F32, kind="ExternalInput")
    t_hprev = nc.dram_tensor("h_prev", [BPC, N * U], F32, kind="ExternalInput")
    t_wk = {g: nc.dram_tensor(f"{g}_kernel", [F * 3, U], F32, kind="ExternalInput")
            for g in "ruc"}
    t_wb = {g: nc.dram_tensor(f"{g}_bias", [U], F32, kind="ExternalInput")
            for g in "ruc"}
    t_out = nc.dram_tensor("out", [BPC, N * U], F32, kind="ExternalOutput")

    QC = 2                 # i-tiles per load chunk (2 MB f32 per chunk)
    NCH = N // (QC * P)    # 4 chunks per batch
    NQ = N // NCH          # 256-column phase chunks
    JPC = JB // NCH        # j-blocks per phase chunk

    with tile.TileContext(nc) as tc:
        with (
            tc.tile_pool(name="const", bufs=1) as constp,
            tc.tile_pool(name="wt", bufs=1) as wtp,
            tc.tile_pool(name="abf", bufs=3) as abfp,
            tc.tile_pool(name="at", bufs=2) as atp,
            tc.tile_pool(name="act", bufs=2) as actp,
            tc.tile_pool(name="psA", bufs=2, space="PSUM") as psA,
            tc.tile_pool(name="psB", bufs=2, space="PSUM") as psB,
        ):
            sup4 = t_supports.ap().rearrange(
                "b (q p) j two -> b p q (j two)", p=P)
            abts = {}

            def load_chunk(b, ch):
                ab = abfp.tile([P, QC * N * S], F32, tag="abf", name="ab",
                               bufs=3)
                nc.gpsimd.dma_start(
                    ab[:], sup4[b, :, ch * QC:(ch + 1) * QC, :])
                abts[(b, ch)] = ab

            # ---- constants ----
            id_bf = constp.tile([P, P], BF16, tag="id_bf")
            make_identity(nc, id_bf[:])
            id_f32 = constp.tile([P, P], F32, tag="id_f32")
            make_identity(nc, id_f32[:])

            # ---- batch-0 x0 load rides the gpsimd ring FIRST so the PE's
            #      opening x0 transposes start as early as possible ----
            G_ = G
            x0ns = []
            x0n0 = actp.tile([P, G_ * F], F32, tag="x0n", bufs=BPC,
                             name="x0n0")
            nc.gpsimd.dma_start(
                x0n0[:].rearrange("p (g f) -> p g f", f=F)[:, :, 0:U],
                t_hprev.ap()[0].rearrange("(p g u) -> p g u", p=P, u=U))
            nc.gpsimd.dma_start(
                x0n0[:].rearrange("p (g f) -> p g f", f=F)[:, :, U:F],
                t_inputs.ap()[0].rearrange("(p g) d -> p g d", p=P))
            x0ns.append(x0n0)

            # ---- gate weights staged via the gpsimd ring (ahead of the
            #      supports chunks in FIFO order) ----
            wst = {}
            for g in "ruc":
                st = wtp.tile([F, 3 * U], F32, tag=f"wst_{g}", name=f"wst_{g}")
                src = t_wk[g].ap().rearrange("(f three) u -> f (three u)", three=3)
                nc.gpsimd.dma_start(st[0:U, :], src[D:F, :])
                nc.gpsimd.dma_start(st[U:F, :], src[0:D, :])
                wst[g] = st

            def w_block(g, m):
                return wst[g][:, m * U:(m + 1) * U]

            w0ru = wtp.tile([F, 2 * U], BF16, tag="w0ru")
            nc.vector.tensor_copy(w0ru[:, 0:U], w_block("r", 0))
            nc.vector.tensor_copy(w0ru[:, U:2 * U], w_block("u", 0))
            wru_s = []
            for s in range(S):
                w = wtp.tile([F, 2 * U], BF16, tag=f"wru{s}")
                nc.vector.tensor_copy(w[:, 0:U], w_block("r", s + 1))
                nc.vector.tensor_copy(w[:, U:2 * U], w_block("u", s + 1))
                wru_s.append(w)
            wc0 = wtp.tile([F, U], BF16, tag="wc0")
            nc.vector.tensor_copy(wc0[:], w_block("c", 0))
            wc_s = []
            for s in range(S):
                w = wtp.tile([F, U], BF16, tag=f"wcs{s}")
                nc.vector.tensor_copy(w[:], w_block("c", s + 1))
                wc_s.append(w)

            bias = {}
            for g in "ruc":
                bt = wtp.tile([U, 1], F32, tag=f"bias_{g}")
                nc.gpsimd.dma_start(bt[:], t_wb[g].ap().rearrange("(u one) -> u one", one=1))
                bias[g] = bt

            # ---- x0 loads for the remaining batches (contiguous reads) ----
            for b in range(1, BPC):
                x0n = actp.tile([P, G * F], F32, tag="x0n", bufs=BPC,
                                name="x0n")
                nc.sync.dma_start(
                    x0n[:].rearrange("p (g f) -> p g f", f=F)[:, :, 0:U],
                    t_hprev.ap()[b].rearrange("(p g u) -> p g u", p=P, u=U))
                nc.sync.dma_start(
                    x0n[:].rearrange("p (g f) -> p g f", f=F)[:, :, U:F],
                    t_inputs.ap()[b].rearrange("(p g) d -> p g d", p=P))
                x0ns.append(x0n)

            for ch in range(NCH):
                load_chunk(0, ch)

            # ---- x0 transposes + z_ru for ALL batches, up front: this PE
            #      work fills the initial chunk-DMA wait window and keeps
            #      batch boundaries gap-free later ----
            x0Tbs, zrus = [], []
            for b in range(BPC):
                x0Tb = actp.tile([F, N], BF16, tag="x0Tb", bufs=BPC,
                                 name="x0Tb")
                x0Tb_g = x0Tb[:].rearrange("f (p g) -> f g p", g=G)
                for g2 in range(G // 2):
                    px = psA.tile([F, 2 * P], F32, tag="psAx", bufs=2)
                    for q in range(2):
                        nc.tensor.matmul(
                            px[:, q * P:(q + 1) * P],
                            x0ns[b][:, (2 * g2 + q) * F:(2 * g2 + q + 1) * F],
                            id_f32[:], start=(q == 0), stop=(q == 1),
                            is_transpose=True)
                    px3 = px[:].rearrange("f (g p) -> f g p", g=2)
                    nc.scalar.copy(x0Tb_g[:, 2 * g2:2 * g2 + 2, :], px3)
                x0Tbs.append(x0Tb)

                zru = []
                for s in range(S):
                    z = actp.tile([P, JB * 2 * U], BF16, tag=f"zru{s}",
                                  bufs=BPC, name="zru")
                    for jb2 in range(JB // 2):
                        pz = psA.tile([P, 2 * 2 * U], F32, tag="psAx", bufs=2)
                        for q in range(2):
                            nc.tensor.matmul(
                                pz[:, q * 2 * U:(q + 1) * 2 * U],
                                x0Tb[:, (2 * jb2 + q) * P:(2 * jb2 + q + 1) * P],
                                wru_s[s][:], start=(q == 0), stop=(q == 1))
                        nc.scalar.copy(
                            z[:, jb2 * 4 * U:(jb2 + 1) * 4 * U], pz[:])
                    zru.append(z)
                zrus.append(zru)

            for b in range(BPC):
                if b + 1 < BPC:
                    for ch in range(NCH):
                        load_chunk(b + 1, ch)

                at = [atp.tile([P, JB * N], BF16, tag=f"at{s}", name=f"at{s}")
                      for s in range(S)]
                x0Tb = x0Tbs[b]
                zru = zrus[b]

                def load_and_transpose(ch):
                    ab = abts.pop((b, ch))
                    # bf16 view of the f32 chunk: the hi halves (h=1) ARE the
                    # bf16 truncation of A, so the PE streams at bf16 rate
                    # straight from the f32 load.  Four j-blocks chain into
                    # one [128, 1024] bf16 PSUM bank per (s, half): fewer,
                    # larger drains pace the PE less.
                    ab5 = ab[:].bitcast(BF16).rearrange(
                        "p (q j s h) -> p q j s h", q=QC, s=S, h=2)
                    k = 0
                    for s in range(S):
                        for jh in range(2):
                            pt8 = psA.tile([P, 4 * QC * P], BF16, tag="psA",
                                           bufs=2, name="pt8")
                            for j4 in range(4):
                                jb = jh * 4 + j4
                                for q in range(QC):
                                    nc.tensor.matmul(
                                        pt8[:, (j4 * QC + q) * P:
                                            (j4 * QC + q + 1) * P],
                                        ab5[:, q, jb * P:(jb + 1) * P, s, 1],
                                        id_bf[:], is_transpose=True,
                                        start=(j4 == 0 and q == 0),
                                        stop=(j4 == 3 and q == QC - 1))
                            # scatter the four j-blocks to their at[] rows
                            dst = at[s][:].rearrange(
                                "p (jb i) -> p jb i", i=N)[
                                :, jh * 4:(jh + 1) * 4,
                                ch * QC * P:(ch + 1) * QC * P]
                            src = pt8[:].rearrange(
                                "p (jb i) -> p jb i", i=QC * P)
                            if k % 4 == 3:
                                nc.scalar.copy(dst, src)
                            else:
                                nc.vector.tensor_copy(dst, src)
                            k += 1

                rT = actp.tile([U, N], BF16, tag="rT")
                uT = actp.tile([U, N], F32, tag="uT")

                def phase1(ic):
                    # column chunk: only needs chunk ic's transposes
                    p1 = psB.tile([P, NQ], F32, tag="psB", name="p1")
                    k = 0
                    for s in range(S):
                        for jb in range(JB):
                            nc.tensor.matmul(
                                p1[:],
                                zru[s][:, jb * 2 * U:(jb + 1) * 2 * U],
                                at[s][:, jb * N + ic * NQ: jb * N + (ic + 1) * NQ],
                                start=(k == 0), stop=False)
                            k += 1
                    nc.tensor.matmul(
                        p1[:], w0ru[:], x0Tb[:, ic * NQ:(ic + 1) * NQ],
                        start=False, stop=True)
                    nc.scalar.activation(
                        rT[:, ic * NQ:(ic + 1) * NQ], p1[0:U, :],
                        mybir.ActivationFunctionType.Sigmoid, bias=bias["r"][:])
                    nc.scalar.activation(
                        uT[:, ic * NQ:(ic + 1) * NQ], p1[U:2 * U, :],
                        mybir.ActivationFunctionType.Sigmoid, bias=bias["u"][:])

                for ch in range(NCH):
                    load_and_transpose(ch)
                    phase1(ch)

                # ---- x0c^T = [(r * H)^T | inputs^T] (bf16) ----
                x0cT = actp.tile([F, N], BF16, tag="x0cT")
                nc.vector.tensor_copy(x0cT[U:F, :], x0Tb[U:F, :])
                for jb in range(JB):
                    nc.vector.tensor_mul(
                        x0cT[0:U, jb * P:(jb + 1) * P],
                        rT[:, jb * P:(jb + 1) * P],
                        x0Tb[0:U, jb * P:(jb + 1) * P])

                # ---- Z_c_s = x0c @ Wc_{s+1}  (N, 64) bf16 ----
                zc = []
                for s in range(S):
                    z = actp.tile([P, JB * U], BF16, tag=f"zc{s}")
                    for jb2 in range(JB // 2):
                        pz = psA.tile([P, 2 * U], F32, tag="psAx", bufs=2)
                        for q in range(2):
                            nc.tensor.matmul(
                                pz[:, q * U:(q + 1) * U],
                                x0cT[:, (2 * jb2 + q) * P:(2 * jb2 + q + 1) * P],
                                wc_s[s][:], start=(q == 0), stop=(q == 1))
                        nc.vector.tensor_copy(
                            z[:, jb2 * 2 * U:(jb2 + 1) * 2 * U], pz[:])
                    zc.append(z)

                # ---- phase 2 + h finalization; the transpose-back matmuls
                #      trail one chunk behind so the PE never waits on the
                #      DVE h-chain ----
                cT = actp.tile([U, N], F32, tag="cT")
                hT = actp.tile([U, N], F32, tag="hT")
                hnat = actp.tile([P, JB * U], F32, tag="hnat")

                def hfin(ic):
                    # h^T = c^T + u^T * (H^T - c^T);  back to natural + store
                    cs = slice(ic * NQ, (ic + 1) * NQ)
                    nc.vector.tensor_sub(hT[:, cs], x0Tb[0:U, cs], cT[:, cs])
                    nc.vector.tensor_mul(hT[:, cs], hT[:, cs], uT[:, cs])
                    nc.vector.tensor_add(hT[:, cs], hT[:, cs], cT[:, cs])

                def hback(ic):
                    for jb2 in range(ic * JPC // 2, (ic + 1) * JPC // 2):
                        ph = psA.tile([P, 2 * U], F32, tag="psAx", bufs=2)
                        for q in range(2):
                            nc.tensor.matmul(
                                ph[:, q * U:(q + 1) * U],
                                hT[:, (2 * jb2 + q) * P:(2 * jb2 + q + 1) * P],
                                id_f32[0:U, 0:U], start=(q == 0), stop=(q == 1),
                                is_transpose=True)
                        nc.vector.tensor_copy(
                            hnat[:, jb2 * 2 * U:(jb2 + 1) * 2 * U], ph[:])
                    nc.sync.dma_start(
                        t_out.ap()[b].rearrange(
                            "(jb p u) -> p jb u", p=P, u=U)[:, ic * JPC:(ic + 1) * JPC, :],
                        hnat[:].rearrange(
                            "p (jb u) -> p jb u", u=U)[:, ic * JPC:(ic + 1) * JPC, :])

                def phase2(ic):
                    p2 = psB.tile([U, NQ], F32, tag="psB2", name="p2")
                    k = 0
                    for s in range(S):
                        for jb in range(JB):
                            nc.tensor.matmul(
                                p2[:],
                                zc[s][:, jb * U:(jb + 1) * U],
                                at[s][:, jb * N + ic * NQ: jb * N + (ic + 1) * NQ],
                                start=(k == 0), stop=False)
                            k += 1
                    nc.tensor.matmul(
                        p2[:], wc0[:], x0cT[:, ic * NQ:(ic + 1) * NQ],
                        start=False, stop=True)
                    nc.scalar.activation(
                        cT[:, ic * NQ:(ic + 1) * NQ], p2[:],
                        mybir.ActivationFunctionType.Tanh, bias=bias["c"][:])
                    hfin(ic)

                for ic in range(NCH):
                    phase2(ic)
                    if ic > 0:
                        hback(ic - 1)
                hback(NCH - 1)

    nc.finalize()
    return nc


def _make_in_maps(inputs):
    in_maps = []
    for c in range(NCORES):
        lo, hi = c * BPC, (c + 1) * BPC
        in_maps.append({
            "inputs": np.ascontiguousarray(inputs["inputs"][lo:hi], np.float32),
            "supports": np.ascontiguousarray(inputs["supports"][lo:hi], np.float32),
            "h_prev": np.ascontiguousarray(inputs["h_prev"][lo:hi], np.float32),
            "r_kernel": np.ascontiguousarray(inputs["r_kernel"], np.float32),
            "u_kernel": np.ascontiguousarray(inputs["u_kernel"], np.float32),
            "c_kernel": np.ascontiguousarray(inputs["c_kernel"], np.float32),
            "r_bias": np.ascontiguousarray(inputs["r_bias"], np.float32),
            "u_bias": np.ascontiguousarray(inputs["u_bias"], np.float32),
            "c_bias": np.ascontiguousarray(inputs["c_bias"], np.float32),
        })
    return in_maps


def kernel(**inputs):
    nc = _COMPILED.get("nc")
    if nc is None:
        nc = _COMPILED["nc"] = _build()

    in_maps = _make_in_maps(inputs)
    last_err = None
    for _ in range(3):
        try:
            res = run_bass_kernel_spmd(nc, in_maps, core_ids=list(range(NCORES)))
            out = np.concatenate(
                [np.asarray(res.results[c]["out"]) for c in range(NCORES)], axis=0)
            return out.astype(np.float32)
        except Exception as e:  # sporadic NRT_EXEC_UNIT_UNRECOVERABLE flakes
            last_err = e
    raise last_err
